# revision 32
# baseline (speedup 1.0000x reference)
"""Sparse GQA attention (nn_MHA_13950053777893) on 8 TRN2 NeuronCores.

Problem: B=2, Sq=Sk=2048, H=16 q-heads, Hkv=4, D=128, f32.
Reference semantics (prefix-valid key padding mask of length sk per batch):
  - score(t, s) = q.k/sqrt(D) for s <= t + sk - Sq, else exactly -10000
  - softmax over s; for rows t < Sq - sk every score is -10000 -> uniform
    attention = mean over ALL Sk value rows (host fills those rows).
  - exp(-10000 - max) == 0 exactly in f32, so softmax over only the
    causally-allowed band matches the reference's full-row softmax for
    rows with a non-empty band.

Sharding (no collectives, disjoint outputs):
  core c in 0..7: kv group g = c // 2, heads {4g + 2*(c%2), 4g + 2*(c%2) + 1}
  for BOTH batches -> each core does 2 heads x 2 batches = 4 head-instances
  and needs only kv head g. Work is identical across cores.

Device algorithm per head-instance (all matmuls bf16 -> f32 PSUM):
  for each 512-wide t-chunk:
    for each 128-row s-block i whose band intersects the chunk:
      tstart = max(t0, 128*floor((s0 + lo)/128))  # band-aligned start
      S^T_psum[s, t] = K^T_i.T @ Q^T[:, tstart:t0+512]     (PE)
      P^T = exp(S^T / sqrt(D)) -> bf16 SBUF                (ACT)
      diagonal region: P^T = affine_select(P^T, 0)         (GPSIMD)
      for each live 128-wide t-sub-block j:
        po_j[t, 0:129] += P^T-slice.T @ [V_i | 1]          (PE, accumulate)
      (po_j column 128 is the softmax denominator for free)
    per live j: rec = 1/po_j[:,128] (DVE), stn = po_j[:,0:128]*rec (DVE)
    one DMA of stn -> out[t, d]   (already in [t, d] layout, no transpose)
"""

import functools

import numpy as np

B, SQ, SK, H, HKV, D = 2, 2048, 2048, 16, 4, 128
CH = 512  # t-chunk width
N_CORES = 8


@functools.lru_cache(maxsize=4)
def _build(sk_tuple):
    import concourse.bass as bass  # noqa: F401
    import concourse.mybir as mybir
    from concourse.tile import TileContext
    from concourse import bacc

    BF16 = mybir.dt.bfloat16
    F32 = mybir.dt.float32
    sks = list(sk_tuple)

    nc = bacc.Bacc(target_bir_lowering=False, debug=False)
    qt_d = nc.dram_tensor("qt", [B, 2, D, SQ], BF16, kind="ExternalInput")
    kt_d = nc.dram_tensor("kt", [B, D, SK], BF16, kind="ExternalInput")
    vo_d = nc.dram_tensor("vo", [B, 128, SK // 128, D + 1], BF16, kind="ExternalInput")
    out_d = nc.dram_tensor("out", [B, 2, SQ, D], BF16, kind="ExternalOutput")

    scale = float(1.0 / np.sqrt(D))
    NSUB = CH // 128

    with TileContext(nc) as tc:
        with (
            tc.tile_pool(name="big", bufs=1) as big,
            tc.tile_pool(name="pt", bufs=14) as ptp,
            tc.tile_pool(name="rec", bufs=8) as recp,
            tc.tile_pool(name="stn", bufs=8) as stp,
            tc.tile_pool(name="psS", bufs=3, space="PSUM") as psS,
            tc.tile_pool(name="psO", bufs=5, space="PSUM") as psO,
        ):
            # critical first Q piece for (b0,h0) chunk t0=512, issued as the
            # very first gpsimd op so its ~2us DMA latency starts early.
            qt00 = big.tile([D, SQ], BF16, tag="qt00", name="qt00")
            nc.gpsimd.dma_start(out=qt00[:, 512:1024], in_=qt_d[0, 0][:, 512:1024])

            # PE warmup: dependency-free matmuls during the DMA prologue keep
            # HAM from throttling the PE when real matmuls start. The operand
            # is memset on-device (a DMA'd operand would make the warmup WAIT
            # for the DMA queue and delay the first real matmul by ~4us).
            ident = big.tile([128, 128], BF16, tag="ident")
            nc.gpsimd.memset(ident, 1.0)
            pw = psS.tile([128, CH], F32, tag="ps", name="pw")
            for _ in range(18):
                nc.tensor.matmul(pw[:, :128], ident, ident, start=True, stop=True)

            kt = {}
            vo = {}
            for b in range(B):
                kt[b] = big.tile([D, SK], BF16, tag=f"kt{b}", name=f"kt{b}")
                if b == 0:
                    # the first chunk's first matmul needs only kt[:, :128]
                    # and qt[:, 512:1024]: tiny critical pieces issued from
                    # otherwise-idle engines so compute starts ~3us earlier.
                    nc.scalar.dma_start(out=kt[b][:, :128], in_=kt_d[b][:, :128])
                    nc.sync.dma_start(out=kt[b][:, 128:512], in_=kt_d[b][:, 128:512])
                    nc.sync.dma_start(out=kt[b][:, 512 : SK // 2], in_=kt_d[b][:, 512 : SK // 2])
                else:
                    nc.sync.dma_start(out=kt[b][:, : SK // 2], in_=kt_d[b][:, : SK // 2])
                nc.sync.dma_start(out=kt[b][:, SK // 2 :], in_=kt_d[b][:, SK // 2 :])
                sk = sks[b]
                lo = SQ - sk  # first row with a non-empty band
                nsb_total = (sk + 127) // 128
                for hh in range(2):
                    if b == 0 and hh == 0:
                        qt = qt00  # cols 512:1024 already in flight (gpsimd)
                        nc.sync.dma_start(out=qt[:, :512], in_=qt_d[b, hh][:, :512])
                        nc.sync.dma_start(out=qt[:, 1024:], in_=qt_d[b, hh][:, 1024:])
                    else:
                        qt = big.tile([D, SQ], BF16, tag=f"qt{b}{hh}")
                        nc.sync.dma_start(out=qt[:, : SQ // 2], in_=qt_d[b, hh][:, : SQ // 2])
                        nc.sync.dma_start(out=qt[:, SQ // 2 :], in_=qt_d[b, hh][:, SQ // 2 :])
                    if b not in vo:
                        vo[b] = big.tile(
                            [128, SK // 128, D + 1], BF16, tag=f"vo{b}", name=f"vo{b}"
                        )
                        nc.sync.dma_start(
                            out=vo[b][:, : SK // 256, :], in_=vo_d[b][:, : SK // 256, :]
                        )
                        nc.sync.dma_start(
                            out=vo[b][:, SK // 256 :, :], in_=vo_d[b][:, SK // 256 :, :]
                        )
                    oview = out_d[b, hh].rearrange("(j p) d -> p j d", p=128)
                    for t0 in range(0, SQ, CH):
                        tend = min(t0 + CH, SQ)
                        nsub = (tend - t0) // 128
                        if tend - 1 < lo:
                            continue  # fully uniform rows; host fills
                        # s-blocks whose band intersects this chunk
                        sblocks = []
                        for i in range(nsb_total):
                            s0 = 128 * i
                            ts_full = 128 * ((s0 + lo) // 128)
                            if ts_full >= tend:
                                break
                            sblocks.append((i, s0, max(t0, ts_full)))
                        # contributors per t-sub-block
                        contrib = {}
                        for order, (i, s0, tstart) in enumerate(sblocks):
                            for j in range((tstart - t0) // 128, nsub):
                                contrib.setdefault(j, []).append(order)
                        j0 = min(contrib)
                        po = {
                            j: psO.tile([128, 512], F32, tag="po", name=f"po{j}")
                            for j in sorted(contrib)
                        }
                        for order, (i, s0, tstart) in enumerate(sblocks):
                            N = tend - tstart
                            # leading columns with NO valid row (t < s0+lo):
                            # skip them in MM1/ACT; affine_select writes zeros.
                            dskip = max(0, min(s0 + lo - tstart, N - 1))
                            ps = psS.tile([128, CH], F32, tag="ps")
                            nc.tensor.matmul(
                                ps[:, dskip:N],
                                kt[b][:, s0 : s0 + 128],
                                qt[:, tstart + dskip : tend],
                                start=True,
                                stop=True,
                            )
                            pt = ptp.tile([128, CH], BF16, tag="pt")
                            nc.scalar.activation(
                                out=pt[:, dskip:N],
                                in_=ps[:, dskip:N],
                                func=mybir.ActivationFunctionType.Exp,
                                scale=scale,
                            )
                            wm = s0 + lo + 128 - tstart
                            if wm > 0:
                                wm = min(wm, N)
                                # zero entries with (tstart+col) - (s0+p) - lo < 0
                                nc.gpsimd.affine_select(
                                    out=pt[:, :wm],
                                    in_=pt[:, :wm],
                                    compare_op=mybir.AluOpType.is_ge,
                                    fill=0.0,
                                    base=tstart - s0 - lo,
                                    channel_multiplier=-1,
                                    pattern=[[1, wm]],
                                )
                            for j in range((tstart - t0) // 128, nsub):
                                off = t0 + 128 * j - tstart
                                nc.tensor.matmul(
                                    po[j][:, : D + 1],
                                    pt[:, off : off + 128],
                                    vo[b][:, i, :],
                                    start=(order == contrib[j][0]),
                                    stop=(order == contrib[j][-1]),
                                )
                        # normalize per sub-block; DMA out per PAIR of
                        # sub-blocks: spreads the output transfers across the
                        # kernel (the final DMA drain was ~10us when bunched
                        # per chunk) without doubling Sync issue cost.
                        live = sorted(contrib)
                        stn = None
                        for jx, j in enumerate(live):
                            rec = recp.tile([128, 1], F32, tag="rec")
                            nc.vector.reciprocal(rec, po[j][:, D : D + 1])
                            if stn is None:
                                stn = stp.tile([128, 2, 128], BF16, tag="stn")
                                jfirst = j
                            nc.vector.tensor_scalar_mul(
                                stn[:, j - jfirst, :], po[j][:, :D], rec
                            )
                            if j - jfirst == 1 or jx == len(live) - 1:
                                nc.sync.dma_start(
                                    out=oview[:, t0 // 128 + jfirst : t0 // 128 + j + 1, :],
                                    in_=stn[:, : j - jfirst + 1, :],
                                )
                                stn = None
    nc.finalize()
    return nc


def kernel(q, kv, key_padding_mask):
    import ml_dtypes
    from concourse.bass_utils import run_bass_kernel_spmd

    q = np.asarray(q, dtype=np.float32)
    kv = np.asarray(kv, dtype=np.float32)
    kpm = np.asarray(key_padding_mask)
    sks = tuple(int(x) for x in kpm.sum(axis=1))

    nc = _build(sks)

    bf16 = ml_dtypes.bfloat16
    k_all = kv[:, :, 0]  # (B, SK, HKV, D)
    v_all = kv[:, :, 1]

    in_maps = []
    for c in range(N_CORES):
        g, half = c // 2, c % 2
        heads = [4 * g + 2 * half, 4 * g + 2 * half + 1]
        qt = np.ascontiguousarray(
            q[:, :, heads, :].transpose(0, 2, 3, 1)  # (B, 2, D, SQ)
        ).astype(bf16)
        kt = np.ascontiguousarray(k_all[:, :, g, :].transpose(0, 2, 1)).astype(bf16)
        vo = np.ones((B, SK, D + 1), dtype=np.float32)
        vo[:, :, :D] = v_all[:, :, g, :]
        vo = np.ascontiguousarray(
            vo.reshape(B, SK // 128, 128, D + 1).transpose(0, 2, 1, 3)
        ).astype(bf16)
        in_maps.append({"qt": qt, "kt": kt, "vo": vo})

    import os

    trace = bool(os.environ.get("BASS_MHA_TRACE"))
    if trace:
        try:
            import trace_hook  # noqa: F401  (dev-only NTFF hook shim)
        except ImportError:
            trace = False

    res = run_bass_kernel_spmd(
        nc, in_maps, list(range(N_CORES)),
        trace=trace, trace_cores=[0] if trace else None,
    )
    kernel._last_exec_time_ns = res.exec_time_ns
    kernel._last_trace = res.instructions_and_trace

    out = np.empty((B, SQ, H, D), dtype=np.float32)
    for c in range(N_CORES):
        g, half = c // 2, c % 2
        heads = [4 * g + 2 * half, 4 * g + 2 * half + 1]
        r = np.asarray(res.results[c]["out"], dtype=np.float32)  # (B, 2, SQ, D)
        for b in range(B):
            for hh, h in enumerate(heads):
                out[b, :, h, :] = r[b, hh]

    # uniform-attention rows: all scores == -10000 -> mean over ALL value rows
    vm = v_all.mean(axis=1)  # (B, HKV, D)
    for b in range(B):
        lo = SQ - sks[b]
        if lo > 0:
            out[b, :lo, :, :] = vm[b, np.arange(H) // (H // HKV), :][None, :, :]
    return out


kernel._last_exec_time_ns = None
kernel._last_trace = None


# revision 33
# speedup vs baseline: 1.2169x; 1.2169x over previous
"""Sparse GQA attention (nn_MHA_13950053777893) on 8 TRN2 NeuronCores.

Problem: B=2, Sq=Sk=2048, H=16 q-heads, Hkv=4, D=128, f32.
Reference semantics (prefix-valid key padding mask of length sk per batch):
  - score(t, s) = q.k/sqrt(D) for s <= t + sk - Sq, else exactly -10000
  - softmax over s; for rows t < Sq - sk every score is -10000 -> uniform
    attention = mean over ALL Sk value rows (host fills those rows).
  - exp(-10000 - max) == 0 exactly in f32, so softmax over only the
    causally-allowed band matches the reference's full-row softmax for
    rows with a non-empty band.

Sharding (no collectives, disjoint outputs):
  core c in 0..7: kv group g = c // 2, heads {4g + 2*(c%2), 4g + 2*(c%2) + 1}
  for BOTH batches -> each core does 2 heads x 2 batches = 4 head-instances
  and needs only kv head g. Work is identical across cores.

Device algorithm per head-instance (all matmuls bf16 -> f32 PSUM):
  for each 512-wide t-chunk:
    for each 128-row s-block i whose band intersects the chunk:
      tstart = max(t0, 128*floor((s0 + lo)/128))  # band-aligned start
      S^T_psum[s, t] = K^T_i.T @ Q^T[:, tstart:t0+512]     (PE)
      P^T = exp(S^T / sqrt(D)) -> bf16 SBUF                (ACT)
      diagonal region: P^T = affine_select(P^T, 0)         (GPSIMD)
      for each live 128-wide t-sub-block j:
        po_j[t, 0:129] += P^T-slice.T @ [V_i | 1]          (PE, accumulate)
      (po_j column 128 is the softmax denominator for free)
    per live j: rec = 1/po_j[:,128] (DVE), stn = po_j[:,0:128]*rec (DVE)
    one DMA of stn -> out[t, d]   (already in [t, d] layout, no transpose)
"""

import functools

import numpy as np

B, SQ, SK, H, HKV, D = 2, 2048, 2048, 16, 4, 128
CH = 512  # t-chunk width
N_CORES = 8


@functools.lru_cache(maxsize=4)
def _build(sk_tuple):
    import concourse.bass as bass  # noqa: F401
    import concourse.mybir as mybir
    from concourse.tile import TileContext
    from concourse import bacc

    BF16 = mybir.dt.bfloat16
    F32 = mybir.dt.float32
    sks = list(sk_tuple)

    nc = bacc.Bacc(target_bir_lowering=False, debug=False)
    qt_d = nc.dram_tensor("qt", [B, 2, D, SQ], BF16, kind="ExternalInput")
    kt_d = nc.dram_tensor("kt", [B, D, SK], BF16, kind="ExternalInput")
    vo_d = nc.dram_tensor("vo", [B, 128, SK // 128, D + 1], BF16, kind="ExternalInput")
    out_d = nc.dram_tensor("out", [B, 2, SQ, D], BF16, kind="ExternalOutput")

    scale = float(1.0 / np.sqrt(D))
    NSUB = CH // 128

    with TileContext(nc) as tc:
        with (
            tc.tile_pool(name="big", bufs=1) as big,
            tc.tile_pool(name="pt", bufs=14) as ptp,
            tc.tile_pool(name="rec", bufs=8) as recp,
            tc.tile_pool(name="stn", bufs=8) as stp,
            tc.tile_pool(name="psS", bufs=4, space="PSUM") as psS,
            tc.tile_pool(name="psO", bufs=4, space="PSUM") as psO,
        ):
            # critical first Q piece for (b0,h0) chunk t0=512, issued as the
            # very first gpsimd op so its ~2us DMA latency starts early.
            qt00 = big.tile([D, SQ], BF16, tag="qt00", name="qt00")
            nc.gpsimd.dma_start(out=qt00[:, 512:1024], in_=qt_d[0, 0][:, 512:1024])

            # PE warmup: dependency-free matmuls during the DMA prologue keep
            # HAM from throttling the PE when real matmuls start. The operand
            # is memset on-device (a DMA'd operand would make the warmup WAIT
            # for the DMA queue and delay the first real matmul by ~4us).
            ident = big.tile([128, 128], BF16, tag="ident")
            nc.gpsimd.memset(ident, 1.0)
            pw = psS.tile([128, CH], F32, tag="ps", name="pw")
            for _ in range(18):
                nc.tensor.matmul(pw[:, :128], ident, ident, start=True, stop=True)

            kt = {}
            vo = {}
            for b in range(B):
                kt[b] = big.tile([D, SK], BF16, tag=f"kt{b}", name=f"kt{b}")
                if b == 0:
                    # the first chunk's first matmul needs only kt[:, :128]
                    # and qt[:, 512:1024]: tiny critical pieces issued from
                    # otherwise-idle engines so compute starts ~3us earlier.
                    nc.scalar.dma_start(out=kt[b][:, :128], in_=kt_d[b][:, :128])
                    nc.sync.dma_start(out=kt[b][:, 128:512], in_=kt_d[b][:, 128:512])
                    nc.sync.dma_start(out=kt[b][:, 512 : SK // 2], in_=kt_d[b][:, 512 : SK // 2])
                else:
                    nc.sync.dma_start(out=kt[b][:, : SK // 2], in_=kt_d[b][:, : SK // 2])
                nc.sync.dma_start(out=kt[b][:, SK // 2 :], in_=kt_d[b][:, SK // 2 :])
                sk = sks[b]
                lo = SQ - sk  # first row with a non-empty band
                nsb_total = (sk + 127) // 128
                for hh in range(2):
                    if b == 0 and hh == 0:
                        qt = qt00  # cols 512:1024 already in flight (gpsimd)
                        nc.sync.dma_start(out=qt[:, :512], in_=qt_d[b, hh][:, :512])
                        nc.sync.dma_start(out=qt[:, 1024:], in_=qt_d[b, hh][:, 1024:])
                    else:
                        qt = big.tile([D, SQ], BF16, tag=f"qt{b}{hh}")
                        nc.sync.dma_start(out=qt[:, : SQ // 2], in_=qt_d[b, hh][:, : SQ // 2])
                        nc.sync.dma_start(out=qt[:, SQ // 2 :], in_=qt_d[b, hh][:, SQ // 2 :])
                    if b not in vo:
                        vo[b] = big.tile(
                            [128, SK // 128, D + 1], BF16, tag=f"vo{b}", name=f"vo{b}"
                        )
                        nc.sync.dma_start(
                            out=vo[b][:, : SK // 256, :], in_=vo_d[b][:, : SK // 256, :]
                        )
                        nc.sync.dma_start(
                            out=vo[b][:, SK // 256 :, :], in_=vo_d[b][:, SK // 256 :, :]
                        )
                    oview = out_d[b, hh].rearrange("(j p) d -> p j d", p=128)
                    for t0 in range(0, SQ, CH):
                        tend = min(t0 + CH, SQ)
                        nsub = (tend - t0) // 128
                        if tend - 1 < lo:
                            continue  # fully uniform rows; host fills
                        # s-blocks whose band intersects this chunk
                        sblocks = []
                        for i in range(nsb_total):
                            s0 = 128 * i
                            ts_full = 128 * ((s0 + lo) // 128)
                            if ts_full >= tend:
                                break
                            sblocks.append((i, s0, max(t0, ts_full)))
                        # contributors per t-sub-block
                        contrib = {}
                        for order, (i, s0, tstart) in enumerate(sblocks):
                            for j in range((tstart - t0) // 128, nsub):
                                contrib.setdefault(j, []).append(order)
                        j0 = min(contrib)
                        po = {
                            j: psO.tile([128, 512], F32, tag="po", name=f"po{j}")
                            for j in sorted(contrib)
                        }
                        for order, (i, s0, tstart) in enumerate(sblocks):
                            N = tend - tstart
                            # leading columns with NO valid row (t < s0+lo):
                            # skip them in MM1/ACT; affine_select writes zeros.
                            dskip = max(0, min(s0 + lo - tstart, N - 1))
                            ps = psS.tile([128, CH], F32, tag="ps")
                            nc.tensor.matmul(
                                ps[:, dskip:N],
                                kt[b][:, s0 : s0 + 128],
                                qt[:, tstart + dskip : tend],
                                start=True,
                                stop=True,
                            )
                            pt = ptp.tile([128, CH], BF16, tag="pt")
                            nc.scalar.activation(
                                out=pt[:, dskip:N],
                                in_=ps[:, dskip:N],
                                func=mybir.ActivationFunctionType.Exp,
                                scale=scale,
                            )
                            wm = s0 + lo + 128 - tstart
                            if wm > 0:
                                wm = min(wm, N)
                                # zero entries with (tstart+col) - (s0+p) - lo < 0
                                nc.gpsimd.affine_select(
                                    out=pt[:, :wm],
                                    in_=pt[:, :wm],
                                    compare_op=mybir.AluOpType.is_ge,
                                    fill=0.0,
                                    base=tstart - s0 - lo,
                                    channel_multiplier=-1,
                                    pattern=[[1, wm]],
                                )
                            for j in range((tstart - t0) // 128, nsub):
                                off = t0 + 128 * j - tstart
                                nc.tensor.matmul(
                                    po[j][:, : D + 1],
                                    pt[:, off : off + 128],
                                    vo[b][:, i, :],
                                    start=(order == contrib[j][0]),
                                    stop=(order == contrib[j][-1]),
                                )
                        # normalize per sub-block; DMA out per PAIR of
                        # sub-blocks: spreads the output transfers across the
                        # kernel (the final DMA drain was ~10us when bunched
                        # per chunk) without doubling Sync issue cost.
                        live = sorted(contrib)
                        stn = None
                        for jx, j in enumerate(live):
                            rec = recp.tile([128, 1], F32, tag="rec")
                            nc.vector.reciprocal(rec, po[j][:, D : D + 1])
                            if stn is None:
                                stn = stp.tile([128, 2, 128], BF16, tag="stn")
                                jfirst = j
                            nc.vector.tensor_scalar_mul(
                                stn[:, j - jfirst, :], po[j][:, :D], rec
                            )
                            if j - jfirst == 1 or jx == len(live) - 1:
                                nc.sync.dma_start(
                                    out=oview[:, t0 // 128 + jfirst : t0 // 128 + j + 1, :],
                                    in_=stn[:, : j - jfirst + 1, :],
                                )
                                stn = None
    nc.finalize()
    return nc


def kernel(q, kv, key_padding_mask):
    import ml_dtypes
    from concourse.bass_utils import run_bass_kernel_spmd

    q = np.asarray(q, dtype=np.float32)
    kv = np.asarray(kv, dtype=np.float32)
    kpm = np.asarray(key_padding_mask)
    sks = tuple(int(x) for x in kpm.sum(axis=1))

    nc = _build(sks)

    bf16 = ml_dtypes.bfloat16
    k_all = kv[:, :, 0]  # (B, SK, HKV, D)
    v_all = kv[:, :, 1]

    in_maps = []
    for c in range(N_CORES):
        g, half = c // 2, c % 2
        heads = [4 * g + 2 * half, 4 * g + 2 * half + 1]
        qt = np.ascontiguousarray(
            q[:, :, heads, :].transpose(0, 2, 3, 1)  # (B, 2, D, SQ)
        ).astype(bf16)
        kt = np.ascontiguousarray(k_all[:, :, g, :].transpose(0, 2, 1)).astype(bf16)
        vo = np.ones((B, SK, D + 1), dtype=np.float32)
        vo[:, :, :D] = v_all[:, :, g, :]
        vo = np.ascontiguousarray(
            vo.reshape(B, SK // 128, 128, D + 1).transpose(0, 2, 1, 3)
        ).astype(bf16)
        in_maps.append({"qt": qt, "kt": kt, "vo": vo})

    import os

    trace = bool(os.environ.get("BASS_MHA_TRACE"))
    if trace:
        try:
            import trace_hook  # noqa: F401  (dev-only NTFF hook shim)
        except ImportError:
            trace = False

    res = run_bass_kernel_spmd(
        nc, in_maps, list(range(N_CORES)),
        trace=trace, trace_cores=[0] if trace else None,
    )
    kernel._last_exec_time_ns = res.exec_time_ns
    kernel._last_trace = res.instructions_and_trace

    out = np.empty((B, SQ, H, D), dtype=np.float32)
    for c in range(N_CORES):
        g, half = c // 2, c % 2
        heads = [4 * g + 2 * half, 4 * g + 2 * half + 1]
        r = np.asarray(res.results[c]["out"], dtype=np.float32)  # (B, 2, SQ, D)
        for b in range(B):
            for hh, h in enumerate(heads):
                out[b, :, h, :] = r[b, hh]

    # uniform-attention rows: all scores == -10000 -> mean over ALL value rows
    vm = v_all.mean(axis=1)  # (B, HKV, D)
    for b in range(B):
        lo = SQ - sks[b]
        if lo > 0:
            out[b, :lo, :, :] = vm[b, np.arange(H) // (H // HKV), :][None, :, :]
    return out


kernel._last_exec_time_ns = None
kernel._last_trace = None
